# revision 31
# baseline (speedup 1.0000x reference)
"""Mixture-of-Experts (top-2 of 8, SwiGLU experts + shared expert) on 8 Trainium2
NeuronCores.

Strategy (expert-parallel, sparse dispatch):
  * Host computes the router (x @ Wr.T, softmax, top-2, renormalised weights) --
    ~0.03% of the model FLOPs -- and uses it to shard the tokens: core e gets
    the tokens whose top-2 contains expert e (padded to a static capacity CA).
  * Core e holds Wg[e]/Wv[e]/Wo[e] and computes the *weighted* expert output
    w_e * (silu(x@Wg.T) * (x@Wv.T)) @ Wo.T for its gathered tokens.
  * The shared expert is sharded 4-way in H x 2-way in tokens: core c
    computes hidden slice (c % 4)*512 of the shared SwiGLU for token half
    (c // 4); the four H-partials of each half sum to the shared output.
  * Host gathers the per-core partials: scatter-adds the weighted expert
    outputs back to token positions, sums the 8 shared partials, adds the
    (zero) biases, and returns (output, gate_logits, topk_i).

All device matmuls keep features on SBUF partitions and tokens on the free
dim, so no on-device transposes are needed (the host pre-transposes weights
and activations into [chunks, 128, free] layout).
"""

import numpy as np
import ml_dtypes

B, S, D, H, O, E, K = 2, 2048, 1024, 2048, 1024, 8, 2
NT = B * S            # total tokens
P = 128
NCORES = 8
# shared expert is sharded 4-way in H x 2-way in tokens:
# core c computes hidden slice (c % 4) for token half (c // 4)
HGRP, TGRP = 4, 2
HS = H // HGRP        # shared-expert hidden slice per core (512)
TSH = NT // TGRP      # shared-expert tokens per core (2048)
NC = 512              # token chunk = matmul free dim
DC, HC, OC, SC = D // P, H // P, O // P, HS // P

_BF16 = ml_dtypes.bfloat16
_cache = {}
_last_run = None      # (nc, in_maps) -- for external profiling harnesses


def _build(CA):
    """Build + compile the SPMD Bass program for expert capacity CA."""
    import concourse.bacc as bacc
    import concourse.mybir as mybir
    from concourse.tile import TileContext

    dt = mybir.dt.bfloat16
    f32 = mybir.dt.float32
    silu = mybir.ActivationFunctionType.Silu

    nc = bacc.Bacc("TRN2", target_bir_lowering=False, debug=False,
                   num_devices=NCORES)

    NB, NR = CA // NC, CA % NC
    xgb = nc.dram_tensor("xgb", [max(NB, 1), P, DC, NC], dt, kind="ExternalInput")
    if NR:
        xgr = nc.dram_tensor("xgr", [P, DC, NR], dt, kind="ExternalInput")
    xf = nc.dram_tensor("xf", [TSH // NC, P, DC, NC], dt, kind="ExternalInput")
    wbc = nc.dram_tensor("wbc", [P, CA], f32, kind="ExternalInput")
    wg = nc.dram_tensor("wg", [DC, P, H], dt, kind="ExternalInput")
    wv = nc.dram_tensor("wv", [DC, P, H], dt, kind="ExternalInput")
    wo = nc.dram_tensor("wo", [HC, P, O], dt, kind="ExternalInput")
    wgs = nc.dram_tensor("wgs", [P, DC, HS], dt, kind="ExternalInput")
    wvs = nc.dram_tensor("wvs", [P, DC, HS], dt, kind="ExternalInput")
    wos = nc.dram_tensor("wos", [P, SC, O], dt, kind="ExternalInput")
    oexp = nc.dram_tensor("oexp", [OC, P, CA], f32, kind="ExternalOutput")
    oshr = nc.dram_tensor("oshr", [OC, P, TSH], dt, kind="ExternalOutput")

    with TileContext(nc) as tc:
        with (
            tc.tile_pool(name="wpool", bufs=1) as wpool,
            tc.tile_pool(name="xpool", bufs=3) as xpool,
            tc.tile_pool(name="gvpool", bufs=1) as gvpool,
            tc.tile_pool(name="sgpool", bufs=3) as sgpool,
            tc.tile_pool(name="opool", bufs=6) as opool,
            tc.tile_pool(name="pg", bufs=2, space="PSUM") as pgpool,
            tc.tile_pool(name="pv", bufs=2, space="PSUM") as pvpool,
            tc.tile_pool(name="po", bufs=3, space="PSUM") as popool,
        ):
            # shared-expert L1 weights (small) are DMA'd first so the PE can
            # start almost immediately; everything else is streamed in pieces
            # interleaved with the shared segment's loads below.
            wgs_sb = wpool.tile([P, DC, HS], dt)
            wvs_sb = wpool.tile([P, DC, HS], dt)
            wos_sb = wpool.tile([P, SC, O], dt)
            wbc_sb = wpool.tile([P, CA], f32)
            wg_sb = wpool.tile([P, DC, H], dt)
            wv_sb = wpool.tile([P, DC, H], dt)
            wo_sb = wpool.tile([P, HC, O], dt)

            def load_expert_piece(j):
                # stream the remaining weights during shared chunk j; each
                # piece lands well before its first consumer needs it
                if j == 0:
                    nc.sync.dma_start(out=wos_sb[:], in_=wos[:])
                    nc.sync.dma_start(out=wbc_sb[:], in_=wbc[:])
                elif j == 1:
                    for d in range(DC):
                        nc.sync.dma_start(out=wg_sb[:, d, :], in_=wg[d])
                elif j == 2:
                    for d in range(DC):
                        nc.sync.dma_start(out=wv_sb[:, d, :], in_=wv[d])
                elif j == 3:
                    for h in range(HC):
                        nc.sync.dma_start(out=wo_sb[:, h, :], in_=wo[h])

            def swiglu_chunk(x_sb, w1_sb, w2_sb, w3_sb, hc, isl, out_dram,
                             scale_sb, odt):
                # x_sb [P, DC, ncw]; w1/w2 [P, DC, hc*P]; w3 [P, hc, O]
                ncw = x_sb.shape[-1]
                gv_sb = gvpool.tile([P, hc, ncw], dt, tag=f"gv{hc}")
                for h in range(hc):
                    g_ps = pgpool.tile([P, ncw], f32, tag="g")
                    v_ps = pvpool.tile([P, ncw], f32, tag="v")
                    hsl = slice(h * P, (h + 1) * P)
                    for d in range(DC):
                        nc.tensor.matmul(g_ps[:], w1_sb[:, d, hsl],
                                         x_sb[:, d, :],
                                         start=(d == 0), stop=(d == DC - 1))
                    for d in range(DC):
                        nc.tensor.matmul(v_ps[:], w2_sb[:, d, hsl],
                                         x_sb[:, d, :],
                                         start=(d == 0), stop=(d == DC - 1))
                    sg = sgpool.tile([P, ncw], f32, tag="sg")
                    nc.scalar.activation(sg[:], g_ps[:], silu)
                    nc.vector.tensor_mul(gv_sb[:, h, :], v_ps[:], sg[:])
                for oc in range(OC):
                    o_ps = popool.tile([P, ncw], f32, tag="o")
                    osl = slice(oc * P, (oc + 1) * P)
                    for h in range(hc):
                        nc.tensor.matmul(o_ps[:], w3_sb[:, h, osl],
                                         gv_sb[:, h, :],
                                         start=(h == 0), stop=(h == hc - 1))
                    ot = opool.tile([P, ncw], odt, tag="ot")
                    if scale_sb is not None:
                        nc.vector.tensor_mul(ot[:], o_ps[:], scale_sb)
                    else:
                        nc.vector.tensor_copy(ot[:], o_ps[:])
                    nc.sync.dma_start(out=out_dram[oc, :, isl], in_=ot[:])

            # shared segment first: its weights are tiny, so the PE starts
            # almost immediately while the (big) expert weights stream in.
            for j in range(TSH // NC):
                jsl = slice(j * NC, (j + 1) * NC)
                x_sb = xpool.tile([P, DC, NC], dt, tag="x")
                if j == 0:
                    # piece-wise in consumption order: the first L1 matmul
                    # (h0, d0) only needs the d=0 slabs, so the PE starts
                    # after ~0.5MB instead of waiting for the full 3MB
                    for d in range(DC):
                        nc.sync.dma_start(out=x_sb[:, d, :], in_=xf[0, :, d, :])
                        nc.sync.dma_start(out=wgs_sb[:, d, :], in_=wgs[:, d, :])
                    for d in range(DC):
                        nc.sync.dma_start(out=wvs_sb[:, d, :], in_=wvs[:, d, :])
                else:
                    nc.sync.dma_start(out=x_sb[:], in_=xf[j])
                load_expert_piece(j)
                swiglu_chunk(x_sb, wgs_sb, wvs_sb, wos_sb, SC, jsl, oshr,
                             None, dt)
            # expert segment: gathered tokens, weighted by gate. The narrow
            # remainder chunk (CA rounds to 128) runs first so the kernel
            # tail is a full-width, well-overlapped chunk.
            if NR:
                isl = slice(NB * NC, CA)
                x_sb = xpool.tile([P, DC, NR], dt, tag="x")
                nc.sync.dma_start(out=x_sb[:], in_=xgr[:])
                swiglu_chunk(x_sb, wg_sb, wv_sb, wo_sb, HC, isl, oexp,
                             wbc_sb[:, isl], f32)
            for i in range(NB):
                isl = slice(i * NC, (i + 1) * NC)
                x_sb = xpool.tile([P, DC, NC], dt, tag="x")
                nc.sync.dma_start(out=x_sb[:], in_=xgb[i])
                swiglu_chunk(x_sb, wg_sb, wv_sb, wo_sb, HC, isl, oexp,
                             wbc_sb[:, isl], f32)

    nc.compile()
    return nc


def _get_compiled(CA):
    if CA not in _cache:
        _cache[CA] = _build(CA)
    return _cache[CA]


def kernel(x, Wr, br, Wg, Wv, Wo, bo, Wg_s, Wv_s, Wo_s, bo_s):
    global _last_run
    x = np.asarray(x, np.float32)
    Wr = np.asarray(Wr, np.float32)
    br = np.asarray(br, np.float32)
    Wg = np.asarray(Wg, np.float32)
    Wv = np.asarray(Wv, np.float32)
    Wo = np.asarray(Wo, np.float32)
    bo = np.asarray(bo, np.float32)
    Wg_s = np.asarray(Wg_s, np.float32)
    Wv_s = np.asarray(Wv_s, np.float32)
    Wo_s = np.asarray(Wo_s, np.float32)
    bo_s = np.asarray(bo_s, np.float32)

    X = x.reshape(NT, D)

    # ---- router on host (float64 for stable ranking) ----
    lg = X.astype(np.float64) @ Wr.astype(np.float64).T + br.astype(np.float64)
    gate_logits = lg.astype(np.float32).reshape(B, S, E)
    m = lg.max(-1, keepdims=True)
    pw = np.exp(lg - m)
    pw /= pw.sum(-1, keepdims=True)
    order = np.argsort(-pw, axis=-1, kind="stable")   # ties -> lower index first
    topk_i = order[:, :K].astype(np.int32)
    topk_w = np.take_along_axis(pw, order[:, :K], axis=-1)
    wren = topk_w / topk_w.sum(-1, keepdims=True)     # renormalised [NT, K]

    tok, wts = [], []
    for e in range(E):
        rows, cols = np.nonzero(topk_i == e)
        tok.append(rows)
        wts.append(wren[rows, cols])
    counts = [len(t) for t in tok]
    CA = max(256, -(-max(counts) // 128) * 128)

    nc = _get_compiled(CA)

    # ---- per-core inputs ----
    Xb = X.astype(_BF16)
    xf_half = [
        np.ascontiguousarray(
            Xb[t * TSH:(t + 1) * TSH].reshape(TSH // NC, NC, DC, P)
            .transpose(0, 3, 2, 1))
        for t in range(TGRP)
    ]
    NB, NR = CA // NC, CA % NC
    in_maps = []
    for e in range(E):
        cnt = counts[e]
        idxp = np.zeros(CA, np.int64)
        idxp[:cnt] = tok[e]
        wp = np.zeros(CA, np.float32)
        wp[:cnt] = wts[e]
        hsl = slice((e % HGRP) * HS, (e % HGRP + 1) * HS)
        Xg_all = Xb[idxp]
        if NB:
            xgb_h = np.ascontiguousarray(
                Xg_all[:NB * NC].reshape(NB, NC, DC, P).transpose(0, 3, 2, 1))
        else:
            xgb_h = np.zeros((1, P, DC, NC), Xb.dtype)
        m = {
            "xgb": xgb_h,
            "xf": xf_half[e // HGRP],
            "wbc": np.ascontiguousarray(np.broadcast_to(wp, (P, CA))),
            "wg": np.ascontiguousarray(Wg[e].T).astype(_BF16).reshape(DC, P, H),
            "wv": np.ascontiguousarray(Wv[e].T).astype(_BF16).reshape(DC, P, H),
            "wo": np.ascontiguousarray(Wo[e].T).astype(_BF16).reshape(HC, P, O),
            "wgs": np.ascontiguousarray(
                Wg_s[hsl].T.astype(_BF16).reshape(DC, P, HS).transpose(1, 0, 2)),
            "wvs": np.ascontiguousarray(
                Wv_s[hsl].T.astype(_BF16).reshape(DC, P, HS).transpose(1, 0, 2)),
            "wos": np.ascontiguousarray(
                Wo_s[:, hsl].T.astype(_BF16).reshape(SC, P, O).transpose(1, 0, 2)),
        }
        if NR:
            m["xgr"] = np.ascontiguousarray(
                Xg_all[NB * NC:].reshape(NR, DC, P).transpose(2, 1, 0))
        in_maps.append(m)

    from concourse import bass_utils
    res = None
    for attempt in range(3):
        try:
            res = bass_utils.run_bass_kernel_spmd(nc, in_maps,
                                                  core_ids=list(range(NCORES)))
            break
        except Exception:
            if attempt == 2:
                raise
            import time
            time.sleep(2.0)
    _last_run = (nc, in_maps)

    # ---- combine on host ----
    accT = np.zeros((O, NT), np.float64)
    for e in range(E):
        t = e // HGRP
        accT[:, t * TSH:(t + 1) * TSH] += \
            res.results[e]["oshr"].reshape(O, TSH).astype(np.float64)
        oexp_e = res.results[e]["oexp"].reshape(O, CA).astype(np.float64)
        accT[:, tok[e]] += oexp_e[:, :counts[e]]
    out = accT.T

    # bias terms (zero for this problem's inputs, kept for generality)
    sparse = np.zeros((NT, E), np.float64)
    np.put_along_axis(sparse, order[:, :K], wren, axis=-1)
    out = out + sparse @ bo.astype(np.float64) + bo_s.astype(np.float64)

    output = out.astype(np.float32).reshape(B, S, O)
    return output, gate_logits, topk_i.reshape(B, S, K)


# revision 32
# speedup vs baseline: 1.0120x; 1.0120x over previous
"""Mixture-of-Experts (top-2 of 8, SwiGLU experts + shared expert) on 8 Trainium2
NeuronCores.

Strategy (expert-parallel, sparse dispatch):
  * Host computes the router (x @ Wr.T, softmax, top-2, renormalised weights) --
    ~0.03% of the model FLOPs -- and uses it to shard the tokens: core e gets
    the tokens whose top-2 contains expert e (padded to a static capacity CA).
  * Core e holds Wg[e]/Wv[e]/Wo[e] and computes the *weighted* expert output
    w_e * (silu(x@Wg.T) * (x@Wv.T)) @ Wo.T for its gathered tokens.
  * The shared expert is sharded 4-way in H x 2-way in tokens: core c
    computes hidden slice (c % 4)*512 of the shared SwiGLU for token half
    (c // 4); the four H-partials of each half sum to the shared output.
  * Host gathers the per-core partials: scatter-adds the weighted expert
    outputs back to token positions, sums the 8 shared partials, adds the
    (zero) biases, and returns (output, gate_logits, topk_i).

All device matmuls keep features on SBUF partitions and tokens on the free
dim, so no on-device transposes are needed (the host pre-transposes weights
and activations into [chunks, 128, free] layout).
"""

import numpy as np
import ml_dtypes

B, S, D, H, O, E, K = 2, 2048, 1024, 2048, 1024, 8, 2
NT = B * S            # total tokens
P = 128
NCORES = 8
# shared expert is sharded 4-way in H x 2-way in tokens:
# core c computes hidden slice (c % 4) for token half (c // 4)
HGRP, TGRP = 4, 2
HS = H // HGRP        # shared-expert hidden slice per core (512)
TSH = NT // TGRP      # shared-expert tokens per core (2048)
NC = 512              # token chunk = matmul free dim
DC, HC, OC, SC = D // P, H // P, O // P, HS // P

_BF16 = ml_dtypes.bfloat16
_cache = {}
_last_run = None      # (nc, in_maps) -- for external profiling harnesses


def _build(CA):
    """Build + compile the SPMD Bass program for expert capacity CA."""
    import concourse.bacc as bacc
    import concourse.mybir as mybir
    from concourse.tile import TileContext

    dt = mybir.dt.bfloat16
    f32 = mybir.dt.float32
    silu = mybir.ActivationFunctionType.Silu

    nc = bacc.Bacc("TRN2", target_bir_lowering=False, debug=False,
                   num_devices=NCORES)

    NB, NR = CA // NC, CA % NC
    xgb = nc.dram_tensor("xgb", [max(NB, 1), P, DC, NC], dt, kind="ExternalInput")
    if NR:
        xgr = nc.dram_tensor("xgr", [P, DC, NR], dt, kind="ExternalInput")
    xf = nc.dram_tensor("xf", [TSH // NC, P, DC, NC], dt, kind="ExternalInput")
    wbc = nc.dram_tensor("wbc", [P, CA], f32, kind="ExternalInput")
    wg = nc.dram_tensor("wg", [DC, P, H], dt, kind="ExternalInput")
    wv = nc.dram_tensor("wv", [DC, P, H], dt, kind="ExternalInput")
    wo = nc.dram_tensor("wo", [HC, P, O], dt, kind="ExternalInput")
    wgs = nc.dram_tensor("wgs", [P, DC, HS], dt, kind="ExternalInput")
    wvs = nc.dram_tensor("wvs", [P, DC, HS], dt, kind="ExternalInput")
    wos = nc.dram_tensor("wos", [P, SC, O], dt, kind="ExternalInput")
    oexp = nc.dram_tensor("oexp", [OC, P, CA], f32, kind="ExternalOutput")
    oshr = nc.dram_tensor("oshr", [OC, P, TSH], dt, kind="ExternalOutput")

    with TileContext(nc) as tc:
        with (
            tc.tile_pool(name="wpool", bufs=1) as wpool,
            tc.tile_pool(name="xpool", bufs=3) as xpool,
            tc.tile_pool(name="gvpool", bufs=1) as gvpool,
            tc.tile_pool(name="sgpool", bufs=3) as sgpool,
            tc.tile_pool(name="opool", bufs=6) as opool,
            tc.tile_pool(name="pg", bufs=2, space="PSUM") as pgpool,
            tc.tile_pool(name="pv", bufs=2, space="PSUM") as pvpool,
            tc.tile_pool(name="po", bufs=3, space="PSUM") as popool,
        ):
            # shared-expert L1 weights (small) are DMA'd first so the PE can
            # start almost immediately; everything else is streamed in pieces
            # interleaved with the shared segment's loads below.
            wgs_sb = wpool.tile([P, DC, HS], dt)
            wvs_sb = wpool.tile([P, DC, HS], dt)
            wos_sb = wpool.tile([P, SC, O], dt)
            wbc_sb = wpool.tile([P, CA], f32)
            wg_sb = wpool.tile([P, DC, H], dt)
            wv_sb = wpool.tile([P, DC, H], dt)
            wo_sb = wpool.tile([P, HC, O], dt)

            def load_expert_piece(j):
                # stream the remaining weights during shared chunk j; each
                # piece lands well before its first consumer needs it
                if j == 0:
                    nc.sync.dma_start(out=wos_sb[:], in_=wos[:])
                    nc.sync.dma_start(out=wbc_sb[:], in_=wbc[:])
                elif j == 1:
                    for d in range(DC):
                        nc.sync.dma_start(out=wg_sb[:, d, :], in_=wg[d])
                elif j == 2:
                    for d in range(DC):
                        nc.sync.dma_start(out=wv_sb[:, d, :], in_=wv[d])
                elif j == 3:
                    for h in range(HC):
                        nc.sync.dma_start(out=wo_sb[:, h, :], in_=wo[h])

            def swiglu_chunk(x_sb, w1_sb, w2_sb, w3_sb, hc, isl, out_dram,
                             scale_sb, odt):
                # x_sb [P, DC, ncw]; w1/w2 [P, DC, hc*P]; w3 [P, hc, O]
                ncw = x_sb.shape[-1]
                gv_sb = gvpool.tile([P, hc, ncw], dt, tag=f"gv{hc}")
                for h in range(hc):
                    g_ps = pgpool.tile([P, ncw], f32, tag="g")
                    v_ps = pvpool.tile([P, ncw], f32, tag="v")
                    hsl = slice(h * P, (h + 1) * P)
                    for d in range(DC):
                        nc.tensor.matmul(g_ps[:], w1_sb[:, d, hsl],
                                         x_sb[:, d, :],
                                         start=(d == 0), stop=(d == DC - 1))
                    for d in range(DC):
                        nc.tensor.matmul(v_ps[:], w2_sb[:, d, hsl],
                                         x_sb[:, d, :],
                                         start=(d == 0), stop=(d == DC - 1))
                    sg = sgpool.tile([P, ncw], f32, tag="sg")
                    nc.scalar.activation(sg[:], g_ps[:], silu)
                    nc.vector.tensor_mul(gv_sb[:, h, :], v_ps[:], sg[:])
                for oc in range(OC):
                    o_ps = popool.tile([P, ncw], f32, tag="o")
                    osl = slice(oc * P, (oc + 1) * P)
                    for h in range(hc):
                        nc.tensor.matmul(o_ps[:], w3_sb[:, h, osl],
                                         gv_sb[:, h, :],
                                         start=(h == 0), stop=(h == hc - 1))
                    ot = opool.tile([P, ncw], odt, tag="ot")
                    if scale_sb is not None:
                        nc.vector.tensor_mul(ot[:], o_ps[:], scale_sb)
                    else:
                        nc.vector.tensor_copy(ot[:], o_ps[:])
                    nc.sync.dma_start(out=out_dram[oc, :, isl], in_=ot[:])

            # shared segment first: its weights are tiny, so the PE starts
            # almost immediately while the (big) expert weights stream in.
            for j in range(TSH // NC):
                jsl = slice(j * NC, (j + 1) * NC)
                x_sb = xpool.tile([P, DC, NC], dt, tag="x")
                nc.sync.dma_start(out=x_sb[:], in_=xf[j])
                if j == 0:
                    nc.sync.dma_start(out=wgs_sb[:], in_=wgs[:])
                    nc.sync.dma_start(out=wvs_sb[:], in_=wvs[:])
                load_expert_piece(j)
                swiglu_chunk(x_sb, wgs_sb, wvs_sb, wos_sb, SC, jsl, oshr,
                             None, dt)
            # expert segment: gathered tokens, weighted by gate. The narrow
            # remainder chunk (CA rounds to 128) runs first so the kernel
            # tail is a full-width, well-overlapped chunk.
            if NR:
                isl = slice(NB * NC, CA)
                x_sb = xpool.tile([P, DC, NR], dt, tag="x")
                nc.sync.dma_start(out=x_sb[:], in_=xgr[:])
                swiglu_chunk(x_sb, wg_sb, wv_sb, wo_sb, HC, isl, oexp,
                             wbc_sb[:, isl], f32)
            for i in range(NB):
                isl = slice(i * NC, (i + 1) * NC)
                x_sb = xpool.tile([P, DC, NC], dt, tag="x")
                nc.sync.dma_start(out=x_sb[:], in_=xgb[i])
                swiglu_chunk(x_sb, wg_sb, wv_sb, wo_sb, HC, isl, oexp,
                             wbc_sb[:, isl], f32)

    nc.compile()
    return nc


def _get_compiled(CA):
    if CA not in _cache:
        _cache[CA] = _build(CA)
    return _cache[CA]


def kernel(x, Wr, br, Wg, Wv, Wo, bo, Wg_s, Wv_s, Wo_s, bo_s):
    global _last_run
    x = np.asarray(x, np.float32)
    Wr = np.asarray(Wr, np.float32)
    br = np.asarray(br, np.float32)
    Wg = np.asarray(Wg, np.float32)
    Wv = np.asarray(Wv, np.float32)
    Wo = np.asarray(Wo, np.float32)
    bo = np.asarray(bo, np.float32)
    Wg_s = np.asarray(Wg_s, np.float32)
    Wv_s = np.asarray(Wv_s, np.float32)
    Wo_s = np.asarray(Wo_s, np.float32)
    bo_s = np.asarray(bo_s, np.float32)

    X = x.reshape(NT, D)

    # ---- router on host (float64 for stable ranking) ----
    lg = X.astype(np.float64) @ Wr.astype(np.float64).T + br.astype(np.float64)
    gate_logits = lg.astype(np.float32).reshape(B, S, E)
    m = lg.max(-1, keepdims=True)
    pw = np.exp(lg - m)
    pw /= pw.sum(-1, keepdims=True)
    order = np.argsort(-pw, axis=-1, kind="stable")   # ties -> lower index first
    topk_i = order[:, :K].astype(np.int32)
    topk_w = np.take_along_axis(pw, order[:, :K], axis=-1)
    wren = topk_w / topk_w.sum(-1, keepdims=True)     # renormalised [NT, K]

    tok, wts = [], []
    for e in range(E):
        rows, cols = np.nonzero(topk_i == e)
        tok.append(rows)
        wts.append(wren[rows, cols])
    counts = [len(t) for t in tok]
    CA = max(256, -(-max(counts) // 128) * 128)

    nc = _get_compiled(CA)

    # ---- per-core inputs ----
    Xb = X.astype(_BF16)
    xf_half = [
        np.ascontiguousarray(
            Xb[t * TSH:(t + 1) * TSH].reshape(TSH // NC, NC, DC, P)
            .transpose(0, 3, 2, 1))
        for t in range(TGRP)
    ]
    NB, NR = CA // NC, CA % NC
    in_maps = []
    for e in range(E):
        cnt = counts[e]
        idxp = np.zeros(CA, np.int64)
        idxp[:cnt] = tok[e]
        wp = np.zeros(CA, np.float32)
        wp[:cnt] = wts[e]
        hsl = slice((e % HGRP) * HS, (e % HGRP + 1) * HS)
        Xg_all = Xb[idxp]
        if NB:
            xgb_h = np.ascontiguousarray(
                Xg_all[:NB * NC].reshape(NB, NC, DC, P).transpose(0, 3, 2, 1))
        else:
            xgb_h = np.zeros((1, P, DC, NC), Xb.dtype)
        m = {
            "xgb": xgb_h,
            "xf": xf_half[e // HGRP],
            "wbc": np.ascontiguousarray(np.broadcast_to(wp, (P, CA))),
            "wg": np.ascontiguousarray(Wg[e].T).astype(_BF16).reshape(DC, P, H),
            "wv": np.ascontiguousarray(Wv[e].T).astype(_BF16).reshape(DC, P, H),
            "wo": np.ascontiguousarray(Wo[e].T).astype(_BF16).reshape(HC, P, O),
            "wgs": np.ascontiguousarray(
                Wg_s[hsl].T.astype(_BF16).reshape(DC, P, HS).transpose(1, 0, 2)),
            "wvs": np.ascontiguousarray(
                Wv_s[hsl].T.astype(_BF16).reshape(DC, P, HS).transpose(1, 0, 2)),
            "wos": np.ascontiguousarray(
                Wo_s[:, hsl].T.astype(_BF16).reshape(SC, P, O).transpose(1, 0, 2)),
        }
        if NR:
            m["xgr"] = np.ascontiguousarray(
                Xg_all[NB * NC:].reshape(NR, DC, P).transpose(2, 1, 0))
        in_maps.append(m)

    from concourse import bass_utils
    res = None
    for attempt in range(3):
        try:
            res = bass_utils.run_bass_kernel_spmd(nc, in_maps,
                                                  core_ids=list(range(NCORES)))
            break
        except Exception:
            if attempt == 2:
                raise
            import time
            time.sleep(2.0)
    _last_run = (nc, in_maps)

    # ---- combine on host ----
    accT = np.zeros((O, NT), np.float64)
    for e in range(E):
        t = e // HGRP
        accT[:, t * TSH:(t + 1) * TSH] += \
            res.results[e]["oshr"].reshape(O, TSH).astype(np.float64)
        oexp_e = res.results[e]["oexp"].reshape(O, CA).astype(np.float64)
        accT[:, tok[e]] += oexp_e[:, :counts[e]]
    out = accT.T

    # bias terms (zero for this problem's inputs, kept for generality)
    sparse = np.zeros((NT, E), np.float64)
    np.put_along_axis(sparse, order[:, :K], wren, axis=-1)
    out = out + sparse @ bo.astype(np.float64) + bo_s.astype(np.float64)

    output = out.astype(np.float32).reshape(B, S, O)
    return output, gate_logits, topk_i.reshape(B, S, K)


# revision 33
# speedup vs baseline: 1.0451x; 1.0328x over previous
"""Mixture-of-Experts (top-2 of 8, SwiGLU experts + shared expert) on 8 Trainium2
NeuronCores.

Strategy (expert-parallel, sparse dispatch):
  * Host computes the router (x @ Wr.T, softmax, top-2, renormalised weights) --
    ~0.03% of the model FLOPs -- and uses it to shard the tokens: core e gets
    the tokens whose top-2 contains expert e (padded to a static capacity CA).
  * Core e holds Wg[e]/Wv[e]/Wo[e] and computes the *weighted* expert output
    w_e * (silu(x@Wg.T) * (x@Wv.T)) @ Wo.T for its gathered tokens.
  * The shared expert is sharded 4-way in H x 2-way in tokens: core c
    computes hidden slice (c % 4)*512 of the shared SwiGLU for token half
    (c // 4); the four H-partials of each half sum to the shared output.
  * Host gathers the per-core partials: scatter-adds the weighted expert
    outputs back to token positions, sums the 8 shared partials, adds the
    (zero) biases, and returns (output, gate_logits, topk_i).

All device matmuls keep features on SBUF partitions and tokens on the free
dim, so no on-device transposes are needed (the host pre-transposes weights
and activations into [chunks, 128, free] layout).
"""

import numpy as np
import ml_dtypes

B, S, D, H, O, E, K = 2, 2048, 1024, 2048, 1024, 8, 2
NT = B * S            # total tokens
P = 128
NCORES = 8
# shared expert is sharded 4-way in H x 2-way in tokens:
# core c computes hidden slice (c % 4) for token half (c // 4)
HGRP, TGRP = 4, 2
HS = H // HGRP        # shared-expert hidden slice per core (512)
TSH = NT // TGRP      # shared-expert tokens per core (2048)
NC = 512              # token chunk = matmul free dim
DC, HC, OC, SC = D // P, H // P, O // P, HS // P

_BF16 = ml_dtypes.bfloat16


def _chunk_widths(CA):
    """Split CA into chunks, each <=512 and >=128 (so every chunk runs at
    PE stream rate, not the LDWEIGHTS floor), summing exactly to CA."""
    widths = []
    left = CA
    while left:
        if left <= 512:
            widths.append(left)
            break
        if left <= 512 + 127:
            widths.extend([left - 128, 128])
            break
        widths.append(512)
        left -= 512
    return widths
_cache = {}
_last_run = None      # (nc, in_maps) -- for external profiling harnesses


def _build(CA):
    """Build + compile the SPMD Bass program for expert capacity CA."""
    import concourse.bacc as bacc
    import concourse.mybir as mybir
    from concourse.tile import TileContext

    dt = mybir.dt.bfloat16
    f32 = mybir.dt.float32
    silu = mybir.ActivationFunctionType.Silu

    nc = bacc.Bacc("TRN2", target_bir_lowering=False, debug=False,
                   num_devices=NCORES)

    xg = nc.dram_tensor("xg", [P, DC, CA], dt, kind="ExternalInput")
    xf = nc.dram_tensor("xf", [TSH // NC, P, DC, NC], dt, kind="ExternalInput")
    wbc = nc.dram_tensor("wbc", [P, CA], f32, kind="ExternalInput")
    wg = nc.dram_tensor("wg", [DC, P, H], dt, kind="ExternalInput")
    wv = nc.dram_tensor("wv", [DC, P, H], dt, kind="ExternalInput")
    wo = nc.dram_tensor("wo", [HC, P, O], dt, kind="ExternalInput")
    wgs = nc.dram_tensor("wgs", [P, DC, HS], dt, kind="ExternalInput")
    wvs = nc.dram_tensor("wvs", [P, DC, HS], dt, kind="ExternalInput")
    wos = nc.dram_tensor("wos", [P, SC, O], dt, kind="ExternalInput")
    oexp = nc.dram_tensor("oexp", [OC, P, CA], f32, kind="ExternalOutput")
    oshr = nc.dram_tensor("oshr", [OC, P, TSH], dt, kind="ExternalOutput")

    with TileContext(nc) as tc:
        with (
            tc.tile_pool(name="wpool", bufs=1) as wpool,
            tc.tile_pool(name="xpool", bufs=3) as xpool,
            tc.tile_pool(name="gvpool", bufs=1) as gvpool,
            tc.tile_pool(name="sgpool", bufs=3) as sgpool,
            tc.tile_pool(name="opool", bufs=6) as opool,
            tc.tile_pool(name="pg", bufs=2, space="PSUM") as pgpool,
            tc.tile_pool(name="pv", bufs=2, space="PSUM") as pvpool,
            tc.tile_pool(name="po", bufs=3, space="PSUM") as popool,
        ):
            # shared-expert L1 weights (small) are DMA'd first so the PE can
            # start almost immediately; everything else is streamed in pieces
            # interleaved with the shared segment's loads below.
            wgs_sb = wpool.tile([P, DC, HS], dt)
            wvs_sb = wpool.tile([P, DC, HS], dt)
            wos_sb = wpool.tile([P, SC, O], dt)
            wbc_sb = wpool.tile([P, CA], f32)
            wg_sb = wpool.tile([P, DC, H], dt)
            wv_sb = wpool.tile([P, DC, H], dt)
            wo_sb = wpool.tile([P, HC, O], dt)

            def load_expert_piece(j):
                # stream the remaining weights during shared chunk j; each
                # piece lands well before its first consumer needs it
                if j == 0:
                    nc.sync.dma_start(out=wos_sb[:], in_=wos[:])
                    nc.sync.dma_start(out=wbc_sb[:], in_=wbc[:])
                elif j == 1:
                    for d in range(DC):
                        nc.sync.dma_start(out=wg_sb[:, d, :], in_=wg[d])
                elif j == 2:
                    for d in range(DC):
                        nc.sync.dma_start(out=wv_sb[:, d, :], in_=wv[d])
                elif j == 3:
                    for h in range(HC):
                        nc.sync.dma_start(out=wo_sb[:, h, :], in_=wo[h])

            def swiglu_chunk(x_sb, w1_sb, w2_sb, w3_sb, hc, isl, out_dram,
                             scale_sb, odt):
                # x_sb [P, DC, ncw]; w1/w2 [P, DC, hc*P]; w3 [P, hc, O]
                ncw = x_sb.shape[-1]
                gv_sb = gvpool.tile([P, hc, ncw], dt, tag=f"gv{hc}")
                for h in range(hc):
                    g_ps = pgpool.tile([P, ncw], f32, tag="g")
                    v_ps = pvpool.tile([P, ncw], f32, tag="v")
                    hsl = slice(h * P, (h + 1) * P)
                    for d in range(DC):
                        nc.tensor.matmul(g_ps[:], w1_sb[:, d, hsl],
                                         x_sb[:, d, :],
                                         start=(d == 0), stop=(d == DC - 1))
                    for d in range(DC):
                        nc.tensor.matmul(v_ps[:], w2_sb[:, d, hsl],
                                         x_sb[:, d, :],
                                         start=(d == 0), stop=(d == DC - 1))
                    sg = sgpool.tile([P, ncw], f32, tag="sg")
                    nc.scalar.activation(sg[:], g_ps[:], silu)
                    nc.vector.tensor_mul(gv_sb[:, h, :], v_ps[:], sg[:])
                for oc in range(OC):
                    o_ps = popool.tile([P, ncw], f32, tag="o")
                    osl = slice(oc * P, (oc + 1) * P)
                    for h in range(hc):
                        nc.tensor.matmul(o_ps[:], w3_sb[:, h, osl],
                                         gv_sb[:, h, :],
                                         start=(h == 0), stop=(h == hc - 1))
                    ot = opool.tile([P, ncw], odt, tag="ot")
                    if scale_sb is not None:
                        nc.vector.tensor_mul(ot[:], o_ps[:], scale_sb)
                    else:
                        nc.vector.tensor_copy(ot[:], o_ps[:])
                    nc.sync.dma_start(out=out_dram[oc, :, isl], in_=ot[:])

            # shared segment first: its weights are tiny, so the PE starts
            # almost immediately while the (big) expert weights stream in.
            for j in range(TSH // NC):
                jsl = slice(j * NC, (j + 1) * NC)
                x_sb = xpool.tile([P, DC, NC], dt, tag="x")
                nc.sync.dma_start(out=x_sb[:], in_=xf[j])
                if j == 0:
                    nc.sync.dma_start(out=wgs_sb[:], in_=wgs[:])
                    nc.sync.dma_start(out=wvs_sb[:], in_=wvs[:])
                load_expert_piece(j)
                swiglu_chunk(x_sb, wgs_sb, wvs_sb, wos_sb, SC, jsl, oshr,
                             None, dt)
            # expert segment: gathered tokens, weighted by gate. Chunk
            # widths sum exactly to CA (no padding rounding); narrowest
            # first so the kernel tail is a wide, well-overlapped chunk.
            widths = _chunk_widths(CA)
            chunks, pos = [], 0
            for w in widths:
                chunks.append((pos, w))
                pos += w
            for pos, w in sorted(chunks, key=lambda c: c[1]):
                isl = slice(pos, pos + w)
                x_sb = xpool.tile([P, DC, w], dt, tag="x")
                nc.sync.dma_start(out=x_sb[:], in_=xg[:, :, isl])
                swiglu_chunk(x_sb, wg_sb, wv_sb, wo_sb, HC, isl, oexp,
                             wbc_sb[:, isl], f32)

    nc.compile()
    return nc


def _get_compiled(CA):
    if CA not in _cache:
        _cache[CA] = _build(CA)
    return _cache[CA]


def kernel(x, Wr, br, Wg, Wv, Wo, bo, Wg_s, Wv_s, Wo_s, bo_s):
    global _last_run
    x = np.asarray(x, np.float32)
    Wr = np.asarray(Wr, np.float32)
    br = np.asarray(br, np.float32)
    Wg = np.asarray(Wg, np.float32)
    Wv = np.asarray(Wv, np.float32)
    Wo = np.asarray(Wo, np.float32)
    bo = np.asarray(bo, np.float32)
    Wg_s = np.asarray(Wg_s, np.float32)
    Wv_s = np.asarray(Wv_s, np.float32)
    Wo_s = np.asarray(Wo_s, np.float32)
    bo_s = np.asarray(bo_s, np.float32)

    X = x.reshape(NT, D)

    # ---- router on host (float64 for stable ranking) ----
    lg = X.astype(np.float64) @ Wr.astype(np.float64).T + br.astype(np.float64)
    gate_logits = lg.astype(np.float32).reshape(B, S, E)
    m = lg.max(-1, keepdims=True)
    pw = np.exp(lg - m)
    pw /= pw.sum(-1, keepdims=True)
    order = np.argsort(-pw, axis=-1, kind="stable")   # ties -> lower index first
    topk_i = order[:, :K].astype(np.int32)
    topk_w = np.take_along_axis(pw, order[:, :K], axis=-1)
    wren = topk_w / topk_w.sum(-1, keepdims=True)     # renormalised [NT, K]

    tok, wts = [], []
    for e in range(E):
        rows, cols = np.nonzero(topk_i == e)
        tok.append(rows)
        wts.append(wren[rows, cols])
    counts = [len(t) for t in tok]
    CA = max(128, max(counts))

    nc = _get_compiled(CA)

    # ---- per-core inputs ----
    Xb = X.astype(_BF16)
    xf_half = [
        np.ascontiguousarray(
            Xb[t * TSH:(t + 1) * TSH].reshape(TSH // NC, NC, DC, P)
            .transpose(0, 3, 2, 1))
        for t in range(TGRP)
    ]

    in_maps = []
    for e in range(E):
        cnt = counts[e]
        idxp = np.zeros(CA, np.int64)
        idxp[:cnt] = tok[e]
        wp = np.zeros(CA, np.float32)
        wp[:cnt] = wts[e]
        hsl = slice((e % HGRP) * HS, (e % HGRP + 1) * HS)
        Xg_all = Xb[idxp]
        m = {
            "xg": np.ascontiguousarray(
                Xg_all.reshape(CA, DC, P).transpose(2, 1, 0)),
            "xf": xf_half[e // HGRP],
            "wbc": np.ascontiguousarray(np.broadcast_to(wp, (P, CA))),
            "wg": np.ascontiguousarray(Wg[e].T).astype(_BF16).reshape(DC, P, H),
            "wv": np.ascontiguousarray(Wv[e].T).astype(_BF16).reshape(DC, P, H),
            "wo": np.ascontiguousarray(Wo[e].T).astype(_BF16).reshape(HC, P, O),
            "wgs": np.ascontiguousarray(
                Wg_s[hsl].T.astype(_BF16).reshape(DC, P, HS).transpose(1, 0, 2)),
            "wvs": np.ascontiguousarray(
                Wv_s[hsl].T.astype(_BF16).reshape(DC, P, HS).transpose(1, 0, 2)),
            "wos": np.ascontiguousarray(
                Wo_s[:, hsl].T.astype(_BF16).reshape(SC, P, O).transpose(1, 0, 2)),
        }
        in_maps.append(m)

    from concourse import bass_utils
    res = None
    for attempt in range(3):
        try:
            res = bass_utils.run_bass_kernel_spmd(nc, in_maps,
                                                  core_ids=list(range(NCORES)))
            break
        except Exception:
            if attempt == 2:
                raise
            import time
            time.sleep(2.0)
    _last_run = (nc, in_maps)

    # ---- combine on host ----
    accT = np.zeros((O, NT), np.float64)
    for e in range(E):
        t = e // HGRP
        accT[:, t * TSH:(t + 1) * TSH] += \
            res.results[e]["oshr"].reshape(O, TSH).astype(np.float64)
        oexp_e = res.results[e]["oexp"].reshape(O, CA).astype(np.float64)
        accT[:, tok[e]] += oexp_e[:, :counts[e]]
    out = accT.T

    # bias terms (zero for this problem's inputs, kept for generality)
    sparse = np.zeros((NT, E), np.float64)
    np.put_along_axis(sparse, order[:, :K], wren, axis=-1)
    out = out + sparse @ bo.astype(np.float64) + bo_s.astype(np.float64)

    output = out.astype(np.float32).reshape(B, S, O)
    return output, gate_logits, topk_i.reshape(B, S, K)
